# revision 34
# baseline (speedup 1.0000x reference)
"""Trainium2 Bass kernel for nn_ExtractorMLP (gnn_message_passing).

edge-MLP with InstanceNorm over the full edge dimension:
    f12 = [emb[col], emb[row]]            # [E, 128] gather
    h   = relu(instnorm(f12 @ W1 + b1))   # [E, 256]
    h   = relu(instnorm(h @ W2 + b2))     # [E, 64]
    out = h @ W3 + b3                     # [E, 1]

Sharding: edges split evenly across 8 cores (data parallel), emb + weights
replicated. InstanceNorm statistics are all-reduced across cores.

Structure (per core, 100352 edges = 196 tiles x 512 after padding):
  * InstanceNorm is shift-invariant, so b1/b2 cancel and are ignored.
  * Pass 1 gathers emb rows for both edge endpoints (single-index-per-
    partition indirect DMAs - the only gather form this runtime executes
    correctly), accumulates the augmented Gram matrix [f12^T f12 | colsum]
    on the PE, transposes tiles to feature-major and spills f12^T (fp16) to
    DRAM. Layer-1 stats come from the Gram: mean = colsum@W1/E and
    E[x^2] = diag(W1^T G W1)/E, so X1 never needs a stats pass of its own.
    Pad edges gather node 0; their Gram contribution is subtracted with one
    K=1 matmul.
  * 2KB of per-core partial stats are AllReduced; every core then computes
    identical normalization scales.
  * Pass 2 streams f12^T back, runs both matmuls in fp16 with the fused
    norm+relu as activation(Relu, scale=istd, bias=-mean*istd), collects
    layer-2 stats with bn_stats, and spills X2^T (fp16).
  * After a second tiny AllReduce, pass 3 normalizes X2^T and applies W3.
"""

import os
import sys

import numpy as np

for _p in ("/opt/trn_rl_repo",):
    if _p not in sys.path:
        sys.path.insert(0, _p)

import concourse.bacc as bacc
import concourse.bass as bass
import concourse.tile as tile
from concourse import mybir
from concourse.bass_utils import run_bass_kernel_spmd
from concourse.masks import make_identity

F32 = mybir.dt.float32
F16 = mybir.dt.float16
I32 = mybir.dt.int32

N_CORES = 8
N, E, H = 50000, 800000, 64
EC = E // N_CORES          # real edges per core
ET = 512                   # edges per compute tile
NT = 196                   # tiles per core
ECP = NT * ET              # padded edges per core (100352)
NPAD = ECP - EC            # 352
EPS = 1e-5

_cache: dict = {}


def _build_nc():
    nc = bacc.Bacc(
        "TRN2", target_bir_lowering=False, debug=False,
        enable_asserts=False, num_devices=N_CORES,
    )

    emb_d = nc.dram_tensor("emb", [N, H], F32, kind="ExternalInput")
    # host-interleaved: [p, t, 2j+side] = edge_index[side, t*512 + j*128 + p]
    eidx_d = nc.dram_tensor("eidx", [128, NT * 8], I32, kind="ExternalInput")
    w1_d = nc.dram_tensor("w1", [2 * H, 4 * H], F32, kind="ExternalInput")
    w2_d = nc.dram_tensor("w2", [4 * H, H], F32, kind="ExternalInput")
    w3_d = nc.dram_tensor("w3", [H, 1], F32, kind="ExternalInput")
    b3_d = nc.dram_tensor("b3", [1, 1], F32, kind="ExternalInput")
    out_d = nc.dram_tensor("out", [ECP, 1], F32, kind="ExternalOutput")

    with tile.TileContext(nc) as tc:
        _emit(nc, tc, emb_d, eidx_d, w1_d, w2_d, w3_d, b3_d, out_d)
    nc.compile()
    return nc


def _emit(nc, tc, emb_d, eidx_d, w1_d, w2_d, w3_d, b3_d, out_d):
    import contextlib

    ctx = contextlib.ExitStack()
    with ctx:
        const = ctx.enter_context(tc.tile_pool(name="const", bufs=1))
        dram = ctx.enter_context(tc.tile_pool(name="dram", bufs=1, space="DRAM"))

        f12buf = dram.tile([128, NT, ET], F16)   # spilled f12^T
        x2buf = dram.tile([H, NT, ET], F16)      # spilled X2^T (no b2)
        table = dram.tile([N, H], F16)           # fp16 emb (halves gather bytes)

        # ---------------- fp16 emb table build ----------------
        emb_flat = emb_d.ap().rearrange("n d -> (n d)").rearrange(
            "(p f) -> p f", p=128)
        tab_flat = table[:].rearrange("n d -> (n d)").rearrange(
            "(p f) -> p f", p=128)
        CH = 12500
        with tc.tile_pool(name="tbuild", bufs=2) as tb:
            for c in range(2):
                tb32 = tb.tile([128, CH], F32, name="tb32")
                nc.sync.dma_start(out=tb32[:],
                                  in_=emb_flat[:, c * CH:(c + 1) * CH])
                tb16 = tb.tile([128, CH], F16, name="tb16")
                nc.vector.tensor_copy(out=tb16[:], in_=tb32[:])
                nc.sync.dma_start(out=tab_flat[:, c * CH:(c + 1) * CH],
                                  in_=tb16[:])

        # ---------------- constants ----------------
        w1f = const.tile([128, 256], F32)
        nc.sync.dma_start(out=w1f[:], in_=w1_d.ap())
        w1h = const.tile([128, 256], F16)
        nc.vector.tensor_copy(out=w1h[:], in_=w1f[:])

        w2f = const.tile([128, 2, H], F32)
        nc.sync.dma_start(
            out=w2f[:], in_=w2_d.ap().rearrange("(c p) m -> p c m", p=128))
        w2h = const.tile([128, 2, H], F16)
        nc.vector.tensor_copy(out=w2h[:], in_=w2f[:])

        w3h = const.tile([H, 1], F16)
        w3f = const.tile([H, 1], F32)
        nc.sync.dma_start(out=w3f[:], in_=w3_d.ap())
        nc.vector.tensor_copy(out=w3h[:], in_=w3f[:])

        b3s = const.tile([1, 1], F32)
        nc.sync.dma_start(out=b3s[:], in_=b3_d.ap())

        ident = const.tile([128, 128], F16)
        make_identity(nc, ident[:])

        ones_f = const.tile([128, 1], F32)
        nc.vector.memset(ones_f[:], 1.0)
        eps_t = const.tile([128, 1], F32)
        nc.vector.memset(eps_t[:], EPS)

        a1 = const.tile([128, 2], F32)   # istd1 per half
        nb1 = const.tile([128, 2], F32)  # -mean1*istd1 per half
        a2 = const.tile([H, 1], F32)
        nb2 = const.tile([H, 1], F32)

        idx_all = const.tile([128, NT, 8], I32)
        nc.sync.dma_start(
            out=idx_all[:].rearrange("p t j -> p (t j)"), in_=eidx_d.ap())

        stats2 = const.tile([H, NT, 6], F32)

        # ============ PASS 1: gather, Gram, spill f12^T ============
        with tc.tile_pool(name="g_psum", bufs=1, space="PSUM") as gp, \
                tc.tile_pool(name="tp_ps", bufs=2, space="PSUM") as tpp, \
                tc.tile_pool(name="c1pool", bufs=1) as c1p, \
                tc.tile_pool(name="fpool", bufs=3) as fp:
            gram = gp.tile([128, 129], F32, tag="gram")
            c1bufs = [c1p.tile([128, 4, 130], F16, tag=f"c1_{k}",
                               name=f"c1_{k}")
                      for k in range(3)]
            for k in range(3):
                nc.vector.memset(c1bufs[k][:, :, 128: 129], 1.0)

            for t in range(NT):
                c1 = c1bufs[t % 3]
                for j in range(4):
                    for side in range(2):
                        nc.gpsimd.indirect_dma_start(
                            out=c1[:, j, 64 * side: 64 * (side + 1)],
                            out_offset=None,
                            in_=table[:],
                            in_offset=bass.IndirectOffsetOnAxis(
                                ap=idx_all[:, t, 2 * j + side: 2 * j + side + 1],
                                axis=0),
                        )
                f12t = fp.tile([128, ET], F16, tag="f12t")
                for j in range(4):
                    first = (t == 0 and j == 0)
                    nc.tensor.matmul(
                        out=gram[:], lhsT=c1[:, j, 0: 128],
                        rhs=c1[:, j, 0: 129], start=first, stop=False)
                    tp = tpp.tile([128, 128], F16, tag="tp")
                    nc.tensor.transpose(
                        out=tp[:], in_=c1[:, j, 0: 128], identity=ident[:])
                    if j < 2:
                        nc.vector.tensor_copy(
                            out=f12t[:, 128 * j: 128 * (j + 1)], in_=tp[:])
                    else:
                        nc.scalar.activation(
                            out=f12t[:, 128 * j: 128 * (j + 1)], in_=tp[:],
                            func=mybir.ActivationFunctionType.Copy)
                nc.sync.dma_start(out=f12buf[:, t, :], in_=f12t[:])

            # pad correction: pads gathered node 0; subtract their Gram
            # contribution with one K=1 matmul.  Use pad edge at tile NT-1,
            # subtile 2, partition 64 (slot 320 >= 160, so a pad), since
            # matmul operands must start at partition 0/32/64.
            c_last = c1bufs[(NT - 1) % 3]
            padv = const.tile([128, 129], F16)
            nc.scalar.activation(
                out=padv[64: 65, :], in_=c_last[64: 65, 2, 0: 129],
                func=mybir.ActivationFunctionType.Copy, scale=-float(NPAD))
            nc.tensor.matmul(
                out=gram[:], lhsT=c_last[64: 65, 2, 0: 128],
                rhs=padv[64: 65, :], start=False, stop=True)

            # ---- layer-1 stats from Gram ----
            gsb = const.tile([128, 129], F32)
            nc.vector.tensor_copy(out=gsb[:], in_=gram[:])

            with tc.tile_pool(name="s1psum", bufs=1, space="PSUM") as sp, \
                    tc.tile_pool(name="s1tmp", bufs=1) as st:
                pack = const.tile([128, 4], F32)
                for h in range(2):
                    w1hf = w1f[:, 128 * h: 128 * (h + 1)]
                    mlin = sp.tile([128, 1], F32, tag="mlin")
                    nc.tensor.matmul(out=mlin[:], lhsT=w1hf,
                                     rhs=gsb[:, 128: 129], start=True, stop=True)
                    nc.vector.tensor_copy(out=pack[:, h: h + 1], in_=mlin[:])
                    s_ps = sp.tile([128, 128], F32, tag="s_ps")
                    nc.tensor.matmul(out=s_ps[:], lhsT=gsb[:, 0: 128],
                                     rhs=w1hf, start=True, stop=True)
                    t_sb = st.tile([128, 128], F32, tag="t_sb")
                    nc.vector.tensor_mul(out=t_sb[:], in0=w1hf, in1=s_ps[:])
                    q_ps = sp.tile([128, 1], F32, tag="q_ps")
                    nc.tensor.matmul(out=q_ps[:], lhsT=t_sb[:],
                                     rhs=ones_f[:], start=True, stop=True)
                    nc.vector.tensor_copy(out=pack[:, 2 + h: 3 + h], in_=q_ps[:])

                st1_i = dram.tile([128, 4], F32)
                st1_o = dram.tile([128, 4], F32, addr_space="Shared")
                nc.sync.dma_start(out=st1_i[:], in_=pack[:])
                nc.gpsimd.collective_compute(
                    "AllReduce", mybir.AluOpType.add,
                    replica_groups=[list(range(N_CORES))],
                    ins=[st1_i.opt()], outs=[st1_o.opt()],
                )
                packr = const.tile([128, 4], F32)
                nc.sync.dma_start(out=packr[:], in_=st1_o[:])

                ml = st.tile([128, 2], F32, tag="ml")
                nc.scalar.activation(out=ml[:], in_=packr[:, 0: 2],
                                     func=mybir.ActivationFunctionType.Copy,
                                     scale=1.0 / E)
                var1 = st.tile([128, 2], F32, tag="var1")
                nc.scalar.activation(out=var1[:], in_=packr[:, 2: 4],
                                     func=mybir.ActivationFunctionType.Copy,
                                     scale=1.0 / E)
                mlsq = st.tile([128, 2], F32, tag="mlsq")
                nc.vector.tensor_mul(out=mlsq[:], in0=ml[:], in1=ml[:])
                nc.vector.tensor_sub(out=var1[:], in0=var1[:], in1=mlsq[:])
                nc.scalar.activation(out=a1[:], in_=var1[:],
                                     func=mybir.ActivationFunctionType.Sqrt,
                                     bias=eps_t[:])
                nc.vector.reciprocal(out=a1[:], in_=a1[:])
                nc.vector.tensor_mul(out=nb1[:], in0=ml[:], in1=a1[:])
                nc.scalar.mul(out=nb1[:], in_=nb1[:], mul=-1.0)

        # ============ PASS 2: MLP + layer-2 stats ============
        last_x2 = None
        with tc.tile_pool(name="f2pool", bufs=3) as fp2, \
                tc.tile_pool(name="h1pool", bufs=2) as hp, \
                tc.tile_pool(name="x2pool", bufs=3) as xp, \
                tc.tile_pool(name="x1_ps", bufs=1, space="PSUM") as x1p, \
                tc.tile_pool(name="x2_ps", bufs=2, space="PSUM") as x2p:
            for t in range(NT):
                f12t = fp2.tile([128, ET], F16, tag="f12l")
                nc.sync.dma_start(out=f12t[:], in_=f12buf[:, t, :])
                h1 = hp.tile([128, 2, ET], F16, tag="h1")
                for h in range(2):
                    x1 = x1p.tile([128, ET], F32, tag=f"x1_{h}")
                    nc.tensor.matmul(
                        out=x1[:], lhsT=w1h[:, 128 * h: 128 * (h + 1)],
                        rhs=f12t[:], start=True, stop=True)
                    nc.scalar.activation(
                        out=h1[:, h, :], in_=x1[:],
                        func=mybir.ActivationFunctionType.Relu,
                        bias=nb1[:, h: h + 1], scale=a1[:, h: h + 1])
                x2 = x2p.tile([H, ET], F32, tag="x2")
                nc.tensor.matmul(out=x2[:], lhsT=w2h[:, 0, :],
                                 rhs=h1[:, 0, :], start=True, stop=False)
                nc.tensor.matmul(out=x2[:], lhsT=w2h[:, 1, :],
                                 rhs=h1[:, 1, :], start=False, stop=True)
                x2s = xp.tile([H, ET], F16, tag="x2s")
                nc.scalar.activation(
                    out=x2s[:], in_=x2[:],
                    func=mybir.ActivationFunctionType.Copy)
                nc.vector.bn_stats(out=stats2[:, t, :], in_=x2s[:])
                nc.sync.dma_start(out=x2buf[:, t, :], in_=x2s[:])
                if t == NT - 1:
                    last_x2 = x2s

        # ---- layer-2 stats ----
        with tc.tile_pool(name="s2tmp", bufs=1) as st:
            mv2 = st.tile([H, 2], F32)
            nc.vector.bn_aggr(out=mv2[:], in_=stats2[:])
            sums = st.tile([H, 2], F32)  # col0 = sum, col1 = sumsq
            nc.scalar.activation(out=sums[:, 0: 1], in_=mv2[:, 0: 1],
                                 func=mybir.ActivationFunctionType.Copy,
                                 scale=float(ECP))
            msq = st.tile([H, 1], F32)
            nc.vector.tensor_mul(out=msq[:], in0=mv2[:, 0: 1], in1=mv2[:, 0: 1])
            nc.vector.tensor_add(out=msq[:], in0=msq[:], in1=mv2[:, 1: 2])
            nc.scalar.activation(out=sums[:, 1: 2], in_=msq[:],
                                 func=mybir.ActivationFunctionType.Copy,
                                 scale=float(ECP))
            # subtract NPAD identical pad columns (pads sit at slots >= 160
            # of the last tile; use slot 511).
            padc = st.tile([H, 1], F32)
            nc.vector.tensor_copy(out=padc[:], in_=last_x2[:, 511: 512])
            padsq = st.tile([H, 1], F32)
            nc.vector.tensor_mul(out=padsq[:], in0=padc[:], in1=padc[:])
            nc.scalar.mul(out=padc[:], in_=padc[:], mul=-float(NPAD))
            nc.scalar.mul(out=padsq[:], in_=padsq[:], mul=-float(NPAD))
            nc.vector.tensor_add(out=sums[:, 0: 1], in0=sums[:, 0: 1], in1=padc[:])
            nc.vector.tensor_add(out=sums[:, 1: 2], in0=sums[:, 1: 2], in1=padsq[:])

            st2_i = dram.tile([H, 2], F32)
            st2_o = dram.tile([H, 2], F32, addr_space="Shared")
            nc.sync.dma_start(out=st2_i[:], in_=sums[:])
            nc.gpsimd.collective_compute(
                "AllReduce", mybir.AluOpType.add,
                replica_groups=[list(range(N_CORES))],
                ins=[st2_i.opt()], outs=[st2_o.opt()],
            )
            sumr = st.tile([H, 2], F32)
            nc.sync.dma_start(out=sumr[:], in_=st2_o[:])

            m2 = st.tile([H, 1], F32)
            nc.scalar.activation(out=m2[:], in_=sumr[:, 0: 1],
                                 func=mybir.ActivationFunctionType.Copy,
                                 scale=1.0 / E)
            var2 = st.tile([H, 1], F32)
            nc.scalar.activation(out=var2[:], in_=sumr[:, 1: 2],
                                 func=mybir.ActivationFunctionType.Copy,
                                 scale=1.0 / E)
            m2sq = st.tile([H, 1], F32)
            nc.vector.tensor_mul(out=m2sq[:], in0=m2[:], in1=m2[:])
            nc.vector.tensor_sub(out=var2[:], in0=var2[:], in1=m2sq[:])
            nc.scalar.activation(out=a2[:], in_=var2[:],
                                 func=mybir.ActivationFunctionType.Sqrt,
                                 bias=eps_t[: H])
            nc.vector.reciprocal(out=a2[:], in_=a2[:])
            nc.vector.tensor_mul(out=nb2[:], in0=m2[:], in1=a2[:])
            nc.scalar.mul(out=nb2[:], in_=nb2[:], mul=-1.0)

        # ============ PASS 3: norm2 + W3 ============
        with tc.tile_pool(name="x2l", bufs=3) as xlp, \
                tc.tile_pool(name="h2pool", bufs=2) as h2p, \
                tc.tile_pool(name="opool", bufs=3) as op, \
                tc.tile_pool(name="o_ps", bufs=2, space="PSUM") as opp:
            for t in range(NT):
                x2l = xlp.tile([H, ET], F16, tag="x2l")
                nc.sync.dma_start(out=x2l[:], in_=x2buf[:, t, :])
                h2 = h2p.tile([H, ET], F16, tag="h2")
                nc.scalar.activation(
                    out=h2[:], in_=x2l[:],
                    func=mybir.ActivationFunctionType.Relu,
                    bias=nb2[:], scale=a2[:])
                o_ps = opp.tile([1, ET], F32, tag="o_ps")
                nc.tensor.matmul(out=o_ps[:], lhsT=w3h[:], rhs=h2[:],
                                 start=True, stop=True)
                o_sb = op.tile([1, ET], F32, tag="o_sb")
                nc.vector.tensor_scalar(
                    out=o_sb[:], in0=o_ps[:], scalar1=b3s[:], scalar2=None,
                    op0=mybir.AluOpType.add)
                nc.sync.dma_start(
                    out=out_d.ap()[ET * t: ET * (t + 1), :]
                    .rearrange("(s) o -> o s"),
                    in_=o_sb[:],
                )


def _get_nc():
    if "nc" not in _cache:
        _cache["nc"] = _build_nc()
    return _cache["nc"]


def _make_in_maps(emb, edge_index, W1, b1, W2, b2, W3, b3):
    emb = np.ascontiguousarray(np.asarray(emb, dtype=np.float32))
    eidx = np.asarray(edge_index)
    W1 = np.ascontiguousarray(np.asarray(W1, dtype=np.float32))
    W2 = np.ascontiguousarray(np.asarray(W2, dtype=np.float32))
    W3 = np.ascontiguousarray(np.asarray(W3, dtype=np.float32)).reshape(H, 1)
    b3 = np.ascontiguousarray(np.asarray(b3, dtype=np.float32)).reshape(1, 1)

    in_maps = []
    for c in range(N_CORES):
        sl = np.asarray(eidx[:, c * EC: (c + 1) * EC], dtype=np.int32)
        pad = np.zeros((2, NPAD), dtype=np.int32)   # pads gather node 0
        arr = np.concatenate([sl, pad], axis=1)     # [2, ECP]
        # [p, t, 2j+side] = arr[side, t*512 + j*128 + p]
        il = np.ascontiguousarray(
            arr.reshape(2, NT, 4, 128).transpose(3, 1, 2, 0)
            .reshape(128, NT * 8))
        in_maps.append({
            "emb": emb, "eidx": il,
            "w1": W1, "w2": W2, "w3": W3, "b3": b3,
        })
    return in_maps


def _gather_out(res):
    outs = []
    for c in range(N_CORES):
        outs.append(res.results[c]["out"][:EC])
    return np.concatenate(outs, axis=0)


def kernel(emb, edge_index, W1, b1, W2, b2, W3, b3):
    in_maps = _make_in_maps(emb, edge_index, W1, b1, W2, b2, W3, b3)
    res = run_bass_kernel_spmd(
        _get_nc(), in_maps, core_ids=list(range(N_CORES)))
    return _gather_out(res)


# revision 36
# speedup vs baseline: 1.0943x; 1.0943x over previous
"""Trainium2 Bass kernel for nn_ExtractorMLP (gnn_message_passing).

edge-MLP with InstanceNorm over the full edge dimension:
    f12 = [emb[col], emb[row]]            # [E, 128] gather
    h   = relu(instnorm(f12 @ W1 + b1))   # [E, 256]
    h   = relu(instnorm(h @ W2 + b2))     # [E, 64]
    out = h @ W3 + b3                     # [E, 1]

Sharding: edges split evenly across 8 cores (data parallel), emb + weights
replicated. InstanceNorm statistics are all-reduced across cores.

Structure (per core, 100352 edges = 196 tiles x 512 after padding):
  * InstanceNorm is shift-invariant, so b1/b2 cancel and are ignored.
  * Pass 1 gathers emb rows for both edge endpoints (single-index-per-
    partition indirect DMAs - the only gather form this runtime executes
    correctly), accumulates the augmented Gram matrix [f12^T f12 | colsum]
    on the PE, transposes tiles to feature-major and spills f12^T (fp16) to
    DRAM. Layer-1 stats come from the Gram: mean = colsum@W1/E and
    E[x^2] = diag(W1^T G W1)/E, so X1 never needs a stats pass of its own.
    Pad edges gather node 0; their Gram contribution is subtracted with one
    K=1 matmul.
  * 2KB of per-core partial stats are AllReduced; every core then computes
    identical normalization scales.
  * Pass 2 streams f12^T back, runs both matmuls in fp16 with the fused
    norm+relu as activation(Relu, scale=istd, bias=-mean*istd), collects
    layer-2 stats with bn_stats, and spills X2^T (fp16).
  * After a second tiny AllReduce, pass 3 normalizes X2^T and applies W3.
"""

import os
import sys

import numpy as np

for _p in ("/opt/trn_rl_repo",):
    if _p not in sys.path:
        sys.path.insert(0, _p)

import concourse.bacc as bacc
import concourse.bass as bass
import concourse.tile as tile
from concourse import mybir
from concourse.bass_utils import run_bass_kernel_spmd
from concourse.masks import make_identity

F32 = mybir.dt.float32
F16 = mybir.dt.float16
I32 = mybir.dt.int32

N_CORES = 8
N, E, H = 50000, 800000, 64
EC = E // N_CORES          # real edges per core
ET = 512                   # edges per compute tile
NT = 196                   # tiles per core
ECP = NT * ET              # padded edges per core (100352)
NPAD = ECP - EC            # 352
EPS = 1e-5

_cache: dict = {}


def _build_nc():
    nc = bacc.Bacc(
        "TRN2", target_bir_lowering=False, debug=False,
        enable_asserts=False, num_devices=N_CORES,
    )

    emb_d = nc.dram_tensor("emb", [N, H], F32, kind="ExternalInput")
    # host-interleaved: [p, t, 2j+side] = edge_index[side, t*512 + j*128 + p]
    eidx_d = nc.dram_tensor("eidx", [128, NT * 8], I32, kind="ExternalInput")
    w1_d = nc.dram_tensor("w1", [2 * H, 4 * H], F32, kind="ExternalInput")
    w2_d = nc.dram_tensor("w2", [4 * H, H], F32, kind="ExternalInput")
    w3_d = nc.dram_tensor("w3", [H, 1], F32, kind="ExternalInput")
    b3_d = nc.dram_tensor("b3", [1, 1], F32, kind="ExternalInput")
    out_d = nc.dram_tensor("out", [ECP, 1], F32, kind="ExternalOutput")

    with tile.TileContext(nc) as tc:
        _emit(nc, tc, emb_d, eidx_d, w1_d, w2_d, w3_d, b3_d, out_d)
    nc.compile()
    return nc


def _emit(nc, tc, emb_d, eidx_d, w1_d, w2_d, w3_d, b3_d, out_d):
    import contextlib

    ctx = contextlib.ExitStack()
    with ctx:
        const = ctx.enter_context(tc.tile_pool(name="const", bufs=1))
        dram = ctx.enter_context(tc.tile_pool(name="dram", bufs=1, space="DRAM"))

        f12buf = dram.tile([128, NT, ET], F16)   # spilled f12^T
        x2buf = dram.tile([H, NT, ET], F16)      # spilled X2^T (no b2)
        table = dram.tile([N, H], F16)           # fp16 emb (halves gather bytes)

        # ---------------- fp16 emb table build ----------------
        emb_flat = emb_d.ap().rearrange("n d -> (n d)").rearrange(
            "(p f) -> p f", p=128)
        tab_flat = table[:].rearrange("n d -> (n d)").rearrange(
            "(p f) -> p f", p=128)
        CH = 12500
        with tc.tile_pool(name="tbuild", bufs=2) as tb:
            for c in range(2):
                tb32 = tb.tile([128, CH], F32, name="tb32")
                nc.sync.dma_start(out=tb32[:],
                                  in_=emb_flat[:, c * CH:(c + 1) * CH])
                tb16 = tb.tile([128, CH], F16, name="tb16")
                nc.vector.tensor_copy(out=tb16[:], in_=tb32[:])
                nc.sync.dma_start(out=tab_flat[:, c * CH:(c + 1) * CH],
                                  in_=tb16[:])

        # ---------------- constants ----------------
        w1f = const.tile([128, 256], F32)
        nc.sync.dma_start(out=w1f[:], in_=w1_d.ap())
        w1h = const.tile([128, 256], F16)
        nc.vector.tensor_copy(out=w1h[:], in_=w1f[:])

        w2f = const.tile([128, 2, H], F32)
        nc.sync.dma_start(
            out=w2f[:], in_=w2_d.ap().rearrange("(c p) m -> p c m", p=128))
        w2h = const.tile([128, 2, H], F16)
        nc.vector.tensor_copy(out=w2h[:], in_=w2f[:])

        w3h = const.tile([H, 1], F16)
        w3f = const.tile([H, 1], F32)
        nc.sync.dma_start(out=w3f[:], in_=w3_d.ap())
        nc.vector.tensor_copy(out=w3h[:], in_=w3f[:])

        b3s = const.tile([1, 1], F32)
        nc.sync.dma_start(out=b3s[:], in_=b3_d.ap())

        ident = const.tile([128, 128], F16)
        make_identity(nc, ident[:])

        ones_f = const.tile([128, 1], F32)
        nc.vector.memset(ones_f[:], 1.0)
        eps_t = const.tile([128, 1], F32)
        nc.vector.memset(eps_t[:], EPS)

        a1 = const.tile([128, 2], F32)   # istd1 per half
        nb1 = const.tile([128, 2], F32)  # -mean1*istd1 per half
        a2 = const.tile([H, 1], F32)
        nb2 = const.tile([H, 1], F32)

        idx_all = const.tile([128, NT, 8], I32)
        nc.sync.dma_start(
            out=idx_all[:].rearrange("p t j -> p (t j)"), in_=eidx_d.ap())

        stats2 = const.tile([H, NT, 6], F32)

        # ============ PASS 1: gather, Gram, spill f12^T ============
        with tc.tile_pool(name="g_psum", bufs=1, space="PSUM") as gp, \
                tc.tile_pool(name="tp_ps", bufs=2, space="PSUM") as tpp, \
                tc.tile_pool(name="c1pool", bufs=1) as c1p, \
                tc.tile_pool(name="fpool", bufs=3) as fp:
            gram = gp.tile([128, 129], F32, tag="gram")
            c1bufs = [c1p.tile([128, 4, 130], F16, tag=f"c1_{k}",
                               name=f"c1_{k}")
                      for k in range(3)]
            for k in range(3):
                nc.vector.memset(c1bufs[k][:, :, 128: 129], 1.0)

            for t in range(NT):
                c1 = c1bufs[t % 3]
                for j in range(4):
                    for side in range(2):
                        nc.gpsimd.indirect_dma_start(
                            out=c1[:, j, 64 * side: 64 * (side + 1)],
                            out_offset=None,
                            in_=table[:],
                            in_offset=bass.IndirectOffsetOnAxis(
                                ap=idx_all[:, t, 2 * j + side: 2 * j + side + 1],
                                axis=0),
                        )
                f12t = fp.tile([128, ET], F16, tag="f12t")
                for j in range(4):
                    first = (t == 0 and j == 0)
                    nc.tensor.matmul(
                        out=gram[:], lhsT=c1[:, j, 0: 128],
                        rhs=c1[:, j, 0: 129], start=first, stop=False)
                    tp = tpp.tile([128, 128], F16, tag="tp")
                    nc.tensor.transpose(
                        out=tp[:], in_=c1[:, j, 0: 128], identity=ident[:])
                    if j < 2:
                        nc.vector.tensor_copy(
                            out=f12t[:, 128 * j: 128 * (j + 1)], in_=tp[:])
                    else:
                        nc.scalar.activation(
                            out=f12t[:, 128 * j: 128 * (j + 1)], in_=tp[:],
                            func=mybir.ActivationFunctionType.Copy)
                nc.sync.dma_start(out=f12buf[:, t, :], in_=f12t[:])

            # pad correction: pads gathered node 0; subtract their Gram
            # contribution with one K=1 matmul.  Use pad edge at tile NT-1,
            # subtile 2, partition 64 (slot 320 >= 160, so a pad), since
            # matmul operands must start at partition 0/32/64.
            c_last = c1bufs[(NT - 1) % 3]
            padv = const.tile([128, 129], F16)
            nc.scalar.activation(
                out=padv[64: 65, :], in_=c_last[64: 65, 2, 0: 129],
                func=mybir.ActivationFunctionType.Copy, scale=-float(NPAD))
            nc.tensor.matmul(
                out=gram[:], lhsT=c_last[64: 65, 2, 0: 128],
                rhs=padv[64: 65, :], start=False, stop=True)

            # ---- layer-1 stats from Gram ----
            gsb = const.tile([128, 129], F32)
            nc.vector.tensor_copy(out=gsb[:], in_=gram[:])

            with tc.tile_pool(name="s1psum", bufs=1, space="PSUM") as sp, \
                    tc.tile_pool(name="s1tmp", bufs=1) as st:
                pack = const.tile([128, 4], F32)
                for h in range(2):
                    w1hf = w1f[:, 128 * h: 128 * (h + 1)]
                    mlin = sp.tile([128, 1], F32, tag="mlin")
                    nc.tensor.matmul(out=mlin[:], lhsT=w1hf,
                                     rhs=gsb[:, 128: 129], start=True, stop=True)
                    nc.vector.tensor_copy(out=pack[:, h: h + 1], in_=mlin[:])
                    s_ps = sp.tile([128, 128], F32, tag="s_ps")
                    nc.tensor.matmul(out=s_ps[:], lhsT=gsb[:, 0: 128],
                                     rhs=w1hf, start=True, stop=True)
                    t_sb = st.tile([128, 128], F32, tag="t_sb")
                    nc.vector.tensor_mul(out=t_sb[:], in0=w1hf, in1=s_ps[:])
                    q_ps = sp.tile([128, 1], F32, tag="q_ps")
                    nc.tensor.matmul(out=q_ps[:], lhsT=t_sb[:],
                                     rhs=ones_f[:], start=True, stop=True)
                    nc.vector.tensor_copy(out=pack[:, 2 + h: 3 + h], in_=q_ps[:])

                st1_i = dram.tile([128, 4], F32)
                st1_o = dram.tile([128, 4], F32, addr_space="Shared")
                nc.sync.dma_start(out=st1_i[:], in_=pack[:])
                nc.gpsimd.collective_compute(
                    "AllReduce", mybir.AluOpType.add,
                    replica_groups=[list(range(N_CORES))],
                    ins=[st1_i.opt()], outs=[st1_o.opt()],
                )
                packr = const.tile([128, 4], F32)
                nc.sync.dma_start(out=packr[:], in_=st1_o[:])

                ml = st.tile([128, 2], F32, tag="ml")
                nc.scalar.activation(out=ml[:], in_=packr[:, 0: 2],
                                     func=mybir.ActivationFunctionType.Copy,
                                     scale=1.0 / E)
                var1 = st.tile([128, 2], F32, tag="var1")
                nc.scalar.activation(out=var1[:], in_=packr[:, 2: 4],
                                     func=mybir.ActivationFunctionType.Copy,
                                     scale=1.0 / E)
                mlsq = st.tile([128, 2], F32, tag="mlsq")
                nc.vector.tensor_mul(out=mlsq[:], in0=ml[:], in1=ml[:])
                nc.vector.tensor_sub(out=var1[:], in0=var1[:], in1=mlsq[:])
                nc.scalar.activation(out=a1[:], in_=var1[:],
                                     func=mybir.ActivationFunctionType.Sqrt,
                                     bias=eps_t[:])
                nc.vector.reciprocal(out=a1[:], in_=a1[:])
                nc.vector.tensor_mul(out=nb1[:], in0=ml[:], in1=a1[:])
                nc.scalar.mul(out=nb1[:], in_=nb1[:], mul=-1.0)

        # ============ PASS 2: MLP + layer-2 stats ============
        last_x2 = None
        with tc.tile_pool(name="f2pool", bufs=3) as fp2, \
                tc.tile_pool(name="h1pool", bufs=2) as hp, \
                tc.tile_pool(name="x2pool", bufs=3) as xp, \
                tc.tile_pool(name="x1_ps", bufs=1, space="PSUM") as x1p, \
                tc.tile_pool(name="x2_ps", bufs=2, space="PSUM") as x2p:
            for t in range(NT):
                f12t = fp2.tile([128, ET], F16, tag="f12l")
                nc.sync.dma_start(out=f12t[:], in_=f12buf[:, t, :])
                h1 = hp.tile([128, 2, ET], F16, tag="h1")
                for h in range(2):
                    x1 = x1p.tile([128, ET], F32, tag=f"x1_{h}")
                    nc.tensor.matmul(
                        out=x1[:], lhsT=w1h[:, 128 * h: 128 * (h + 1)],
                        rhs=f12t[:], start=True, stop=True)
                    nc.scalar.activation(
                        out=h1[:, h, :], in_=x1[:],
                        func=mybir.ActivationFunctionType.Relu,
                        bias=nb1[:, h: h + 1], scale=a1[:, h: h + 1])
                x2 = x2p.tile([H, ET], F32, tag="x2")
                nc.tensor.matmul(out=x2[:], lhsT=w2h[:, 0, :],
                                 rhs=h1[:, 0, :], start=True, stop=False)
                nc.tensor.matmul(out=x2[:], lhsT=w2h[:, 1, :],
                                 rhs=h1[:, 1, :], start=False, stop=True)
                x2s = xp.tile([H, ET], F16, tag="x2s")
                nc.scalar.activation(
                    out=x2s[:], in_=x2[:],
                    func=mybir.ActivationFunctionType.Copy)
                nc.vector.bn_stats(out=stats2[:, t, :], in_=x2s[:])
                nc.sync.dma_start(out=x2buf[:, t, :], in_=x2s[:])
                if t == NT - 1:
                    last_x2 = x2s

        # ---- layer-2 stats ----
        with tc.tile_pool(name="s2tmp", bufs=1) as st:
            mv2 = st.tile([H, 2], F32)
            nc.vector.bn_aggr(out=mv2[:], in_=stats2[:])
            sums = st.tile([H, 2], F32)  # col0 = sum, col1 = sumsq
            nc.scalar.activation(out=sums[:, 0: 1], in_=mv2[:, 0: 1],
                                 func=mybir.ActivationFunctionType.Copy,
                                 scale=float(ECP))
            msq = st.tile([H, 1], F32)
            nc.vector.tensor_mul(out=msq[:], in0=mv2[:, 0: 1], in1=mv2[:, 0: 1])
            nc.vector.tensor_add(out=msq[:], in0=msq[:], in1=mv2[:, 1: 2])
            nc.scalar.activation(out=sums[:, 1: 2], in_=msq[:],
                                 func=mybir.ActivationFunctionType.Copy,
                                 scale=float(ECP))
            # subtract NPAD identical pad columns (pads sit at slots >= 160
            # of the last tile; use slot 511).
            padc = st.tile([H, 1], F32)
            nc.vector.tensor_copy(out=padc[:], in_=last_x2[:, 511: 512])
            padsq = st.tile([H, 1], F32)
            nc.vector.tensor_mul(out=padsq[:], in0=padc[:], in1=padc[:])
            nc.scalar.mul(out=padc[:], in_=padc[:], mul=-float(NPAD))
            nc.scalar.mul(out=padsq[:], in_=padsq[:], mul=-float(NPAD))
            nc.vector.tensor_add(out=sums[:, 0: 1], in0=sums[:, 0: 1], in1=padc[:])
            nc.vector.tensor_add(out=sums[:, 1: 2], in0=sums[:, 1: 2], in1=padsq[:])

            st2_i = dram.tile([H, 2], F32)
            st2_o = dram.tile([H, 2], F32, addr_space="Shared")
            nc.sync.dma_start(out=st2_i[:], in_=sums[:])
            nc.gpsimd.collective_compute(
                "AllReduce", mybir.AluOpType.add,
                replica_groups=[list(range(N_CORES))],
                ins=[st2_i.opt()], outs=[st2_o.opt()],
            )
            sumr = st.tile([H, 2], F32)
            nc.sync.dma_start(out=sumr[:], in_=st2_o[:])

            m2 = st.tile([H, 1], F32)
            nc.scalar.activation(out=m2[:], in_=sumr[:, 0: 1],
                                 func=mybir.ActivationFunctionType.Copy,
                                 scale=1.0 / E)
            var2 = st.tile([H, 1], F32)
            nc.scalar.activation(out=var2[:], in_=sumr[:, 1: 2],
                                 func=mybir.ActivationFunctionType.Copy,
                                 scale=1.0 / E)
            m2sq = st.tile([H, 1], F32)
            nc.vector.tensor_mul(out=m2sq[:], in0=m2[:], in1=m2[:])
            nc.vector.tensor_sub(out=var2[:], in0=var2[:], in1=m2sq[:])
            nc.scalar.activation(out=a2[:], in_=var2[:],
                                 func=mybir.ActivationFunctionType.Sqrt,
                                 bias=eps_t[: H])
            nc.vector.reciprocal(out=a2[:], in_=a2[:])
            nc.vector.tensor_mul(out=nb2[:], in0=m2[:], in1=a2[:])
            nc.scalar.mul(out=nb2[:], in_=nb2[:], mul=-1.0)

        # ============ PASS 3: norm2 + W3 ============
        with tc.tile_pool(name="x2l", bufs=3) as xlp, \
                tc.tile_pool(name="h2pool", bufs=2) as h2p, \
                tc.tile_pool(name="opool", bufs=3) as op, \
                tc.tile_pool(name="o_ps", bufs=2, space="PSUM") as opp:
            for t in range(NT):
                x2l = xlp.tile([H, ET], F16, tag="x2l")
                nc.sync.dma_start(out=x2l[:], in_=x2buf[:, t, :])
                h2 = h2p.tile([H, ET], F16, tag="h2")
                nc.scalar.activation(
                    out=h2[:], in_=x2l[:],
                    func=mybir.ActivationFunctionType.Relu,
                    bias=nb2[:], scale=a2[:])
                o_ps = opp.tile([1, ET], F32, tag="o_ps")
                nc.tensor.matmul(out=o_ps[:], lhsT=w3h[:], rhs=h2[:],
                                 start=True, stop=True)
                o_sb = op.tile([1, ET], F32, tag="o_sb")
                nc.vector.tensor_scalar(
                    out=o_sb[:], in0=o_ps[:], scalar1=b3s[:], scalar2=None,
                    op0=mybir.AluOpType.add)
                nc.sync.dma_start(
                    out=out_d.ap()[ET * t: ET * (t + 1), :]
                    .rearrange("(s) o -> o s"),
                    in_=o_sb[:],
                )


def _get_nc():
    if "nc" not in _cache:
        _cache["nc"] = _build_nc()
    return _cache["nc"]


def _make_in_maps(emb, edge_index, W1, b1, W2, b2, W3, b3):
    emb = np.ascontiguousarray(np.asarray(emb, dtype=np.float32))
    eidx = np.asarray(edge_index)
    W1 = np.ascontiguousarray(np.asarray(W1, dtype=np.float32))
    W2 = np.ascontiguousarray(np.asarray(W2, dtype=np.float32))
    W3 = np.ascontiguousarray(np.asarray(W3, dtype=np.float32)).reshape(H, 1)
    b3 = np.ascontiguousarray(np.asarray(b3, dtype=np.float32)).reshape(1, 1)

    in_maps = []
    for c in range(N_CORES):
        sl = np.asarray(eidx[:, c * EC: (c + 1) * EC], dtype=np.int32)
        pad = np.zeros((2, NPAD), dtype=np.int32)   # pads gather node 0
        arr = np.concatenate([sl, pad], axis=1)     # [2, ECP]
        # [p, t, 2j+side] = arr[side, t*512 + j*128 + p]
        il = np.ascontiguousarray(
            arr.reshape(2, NT, 4, 128).transpose(3, 1, 2, 0)
            .reshape(128, NT * 8))
        in_maps.append({
            "emb": emb, "eidx": il,
            "w1": W1, "w2": W2, "w3": W3, "b3": b3,
        })
    return in_maps


def _gather_out(res):
    outs = []
    for c in range(N_CORES):
        outs.append(res.results[c]["out"][:EC])
    return np.concatenate(outs, axis=0)


def kernel(emb, edge_index, W1, b1, W2, b2, W3, b3):
    in_maps = _make_in_maps(emb, edge_index, W1, b1, W2, b2, W3, b3)
    res = run_bass_kernel_spmd(
        _get_nc(), in_maps, core_ids=list(range(N_CORES)))
    return _gather_out(res)
